# revision 8
# baseline (speedup 1.0000x reference)
"""Trainium2 Bass kernel for CustomAttention (qkv -> per-head LN on q,k -> SDPA -> proj).

Sharding: 8 cores = 2 batches x 4 head-groups (3 heads each).
Per core: qkv projection for its heads from x[b], full attention per head
(scores computed transposed so no probability-matrix transpose is needed,
softmax denominator folded into the PV matmul as a ones column on V),
then a partial output projection over its 192 channels. Host sums the 4
partials per batch and adds proj_b.
"""

import os
import sys
from functools import lru_cache

import numpy as np

for _p in ("/opt/trn_rl_repo", os.path.expanduser("~/.axon_site/_ro/trn_rl_repo")):
    if os.path.isdir(_p) and _p not in sys.path:
        sys.path.insert(0, _p)

import concourse.bass as bass
import concourse.mybir as mybir
from concourse import bacc
import concourse.tile as tile
from concourse.masks import make_identity

F32 = mybir.dt.float32
F32R = mybir.dt.float32r
ALU = mybir.AluOpType
ACTF = mybir.ActivationFunctionType

H = 3          # heads per core
D = 64         # head dim
C = 768        # model dim
J = 3 * H * D  # qkv rows per core = 576
EPS = 1e-5
SCALE = D ** -0.5

# Phase-C software pipeline skew: PV matmuls run this many exp-tiles behind
# the score matmuls so PE never stalls on the ACT engine.
SKEW = 4


def r32(ap):
    return ap.bitcast(F32R)


def build_nc(N=4096):
    """One-core program; all 8 cores run it SPMD with different input data."""
    NB = N // 128          # n-blocks / j-chunks
    IB = N // 512          # i-blocks
    NHALF = NB // 2        # j-chunk pairs for row-tiled score matmuls

    nc = bacc.Bacc("TRN2", target_bir_lowering=False, debug=False)
    x_t = nc.declare_dram_parameter("x_t", [C, N], F32, isOutput=False)
    wqkv_t = nc.declare_dram_parameter("wqkv_t", [C, J], F32, isOutput=False)
    projw_t = nc.declare_dram_parameter("projw_t", [H * D, C], F32, isOutput=False)
    gb = nc.declare_dram_parameter("gb", [4, D], F32, isOutput=False)
    out_p = nc.declare_dram_parameter("out_p", [N, C], F32, isOutput=True)

    with tile.TileContext(nc) as tc:
        with (
            tc.tile_pool(name="persist", bufs=1) as persist,
            tc.tile_pool(name="weights", bufs=1) as weights,
        ):
            # ---- persistent SBUF tensors ----
            # qT duplicated across both partition halves: rows 0:64 == 64:128
            qT = [persist.tile([128, N], F32R, tag=f"qT{h}", name=f"qT{h}") for h in range(H)]
            # kT stacked: rows 0:64 = j in [0,N/2), rows 64:128 = j in [N/2,N)
            kT = [persist.tile([128, N // 2], F32R, tag=f"kT{h}", name=f"kT{h}") for h in range(H)]
            # V augmented with a ones column (index 64) per j-chunk
            vA = [persist.tile([128, NB, 65], F32R, tag=f"vA{h}", name=f"vA{h}") for h in range(H)]
            # attention output, channel-major: ao1 rows = h0,h1; ao2 rows = h2
            ao1 = persist.tile([128, N], F32R, tag="ao1")
            ao2 = persist.tile([64, N], F32R, tag="ao2")

            ident = persist.tile([128, 128], F32, tag="ident")
            make_identity(nc, ident[:])
            ones64 = persist.tile([1, D], F32R, tag="ones64")
            nc.vector.memset(ones64[:].bitcast(F32), 1.0)
            for h in range(H):
                nc.vector.memset(vA[h][:, :, 64:65].bitcast(F32), 1.0)

            wq = weights.tile([128, 6, J], F32R, tag="wqkv")
            nc.sync.dma_start(
                wq[:], wqkv_t.rearrange("(ck p) j -> p ck j", p=128).bitcast(F32R)
            )
            pw128 = weights.tile([128, C], F32R, tag="pw128")
            nc.sync.dma_start(pw128[:], projw_t[0:128, :].bitcast(F32R))
            pw64 = weights.tile([64, C], F32R, tag="pw64")
            nc.sync.dma_start(pw64[:], projw_t[128:192, :].bitcast(F32R))
            # gamma/beta broadcast across partitions: [gq*s, bq*s, gk, bk]
            gbt = weights.tile([128, 4, D], F32, tag="gb")
            epst = weights.tile([128, 1], F32, tag="epst")
            nc.vector.memset(epst[:], EPS)
            nc.sync.dma_start(gbt[:], gb[None, :, :].to_broadcast([128, 4, D]))

            # ================= Phase B: qkv + LN + transpose =================
            with (
                tc.tile_pool(name="pB", bufs=3) as pB,
                tc.tile_pool(name="pBs", bufs=3) as pBs,
                tc.tile_pool(name="psB", bufs=4, space="PSUM") as psB,
                tc.tile_pool(name="psT", bufs=4, space="PSUM") as psT,
            ):
                for nb in range(NB):
                    xt = pB.tile([128, 6, 128], F32R, tag="xt")
                    nc.sync.dma_start(
                        xt[:],
                        x_t.rearrange("(ck p) n -> p ck n", p=128)[
                            :, :, nb * 128 : (nb + 1) * 128
                        ].bitcast(F32R),
                    )
                    qkv = pB.tile([128, J], F32, tag="qkv")
                    for half in range(2):
                        ps = psB.tile([128, 288], F32, tag="qkvps")
                        for ck in range(6):
                            nc.tensor.matmul(
                                ps[:],
                                r32(xt[:, ck, :]),
                                r32(wq[:, ck, half * 288 : (half + 1) * 288]),
                                start=(ck == 0),
                                stop=(ck == 5),
                            )
                        nc.any.tensor_copy(qkv[:, half * 288 : (half + 1) * 288], ps[:])

                    # LayerNorm over d for q (cols 0:192) and k (cols 192:384),
                    # writing a column-duplicated result for the transposes.
                    for t in range(2):  # 0 = q, 1 = k
                        src = qkv[:, t * 192 : (t + 1) * 192]
                        src3 = src.rearrange("p (h d) -> p h d", h=H)
                        s1 = pBs.tile([128, H], F32, tag="s1")
                        nc.vector.tensor_reduce(
                            s1[:], src3, mybir.AxisListType.X, ALU.add
                        )
                        mu = pBs.tile([128, H], F32, tag="mu")
                        nc.vector.tensor_scalar_mul(mu[:], s1[:], 1.0 / D)
                        sq = pBs.tile([128, 192], F32, tag="sq")
                        nc.vector.tensor_mul(sq[:], src, src)
                        s2 = pBs.tile([128, H], F32, tag="s2")
                        nc.vector.tensor_reduce(
                            s2[:],
                            sq.rearrange("p (h d) -> p h d", h=H),
                            mybir.AxisListType.X,
                            ALU.add,
                        )
                        var = pBs.tile([128, H], F32, tag="var")
                        # var = s2/D - mu^2  (computed as (s2/D) - mu*mu)
                        nc.vector.tensor_scalar_mul(var[:], s2[:], 1.0 / D)
                        musq = pBs.tile([128, H], F32, tag="musq")
                        nc.vector.tensor_mul(musq[:], mu[:], mu[:])
                        nc.vector.tensor_sub(var[:], var[:], musq[:])
                        std = pBs.tile([128, H], F32, tag="std")
                        nc.scalar.activation(std[:], var[:], ACTF.Sqrt, bias=epst[:])
                        rstd = pBs.tile([128, H], F32, tag="rstd")
                        nc.vector.reciprocal(rstd[:], std[:])
                        # one Newton step: r <- r*(1.5 - 0.5*(var+eps)*r^2)
                        nr = pBs.tile([128, H], F32, tag="nr")
                        nc.vector.tensor_mul(nr[:], rstd[:], rstd[:])
                        ve = pBs.tile([128, H], F32, tag="ve")
                        nc.vector.tensor_scalar_add(ve[:], var[:], EPS)
                        nc.vector.tensor_mul(nr[:], nr[:], ve[:])
                        nc.vector.tensor_scalar(
                            nr[:], nr[:], -0.5, 1.5, ALU.mult, ALU.add
                        )
                        nc.vector.tensor_mul(rstd[:], rstd[:], nr[:])

                        cs = pBs.tile([128, 192], F32, tag="cs")
                        cs3 = cs.rearrange("p (h d) -> p h d", h=H)
                        nc.vector.tensor_sub(
                            cs3, src3, mu[:, :, None].broadcast_to([128, H, D])
                        )
                        nc.vector.tensor_mul(
                            cs3, cs3, rstd[:, :, None].broadcast_to([128, H, D])
                        )
                        gam = gbt[:, 2 * t, :][:, None, :].broadcast_to([128, H, D])
                        bet = gbt[:, 2 * t + 1, :][:, None, :].broadcast_to([128, H, D])
                        nc.vector.tensor_mul(cs3, cs3, gam)
                        dup = pBs.tile([128, H, 2, D], F32, tag=f"dup{t}")
                        nc.vector.tensor_add(dup[:, :, 0, :], cs3, bet)
                        nc.vector.tensor_add(dup[:, :, 1, :], cs3, bet)

                        # transpose each head's duplicated [128,128] slab
                        dupf = dup[:].rearrange("p h r d -> p (h r d)")
                        for h in range(H):
                            pst = psT.tile([128, 128], F32, tag="pst")
                            nc.tensor.transpose(
                                pst[:], dupf[:, h * 128 : (h + 1) * 128], ident[:]
                            )
                            blk = slice(nb * 128, (nb + 1) * 128)
                            if t == 0:
                                nc.any.tensor_copy(qT[h][:, blk], pst[:])
                            else:
                                jh = nb // NHALF
                                cb = nb % NHALF
                                nc.any.tensor_copy(
                                    kT[h][
                                        64 * jh : 64 * jh + 64,
                                        cb * 128 : (cb + 1) * 128,
                                    ],
                                    pst[64 * jh : 64 * jh + 64, :],
                                )
                    for h in range(H):
                        nc.any.tensor_copy(
                            vA[h][:, nb, 0:64],
                            qkv[:, 384 + h * 64 : 384 + (h + 1) * 64],
                        )

            # ================= Phase C: attention =================
            with (
                tc.tile_pool(name="pt", bufs=2 * SKEW + 2) as ptp,
                tc.tile_pool(name="pCs", bufs=3) as pCs,
                tc.tile_pool(name="psS", bufs=4, space="PSUM") as psS,
                tc.tile_pool(name="psO", bufs=2, space="PSUM") as psO,
                tc.tile_pool(name="psBC", bufs=2, space="PSUM") as psBC,
            ):
                for h in range(H):
                    for ib in range(IB):
                        isl = slice(ib * 512, (ib + 1) * 512)
                        pso = psO.tile([65, 512], F32, tag="pso")
                        queue = []
                        n_pv = [0]

                        def emit_pv(pso=pso, queue=queue, n_pv=n_pv, h=h):
                            pt, jc = queue.pop(0)
                            nc.tensor.matmul(
                                pso[:],
                                r32(vA[h][:, jc, :]),
                                r32(pt[:]),
                                start=(n_pv[0] == 0),
                                stop=(n_pv[0] == NB - 1),
                            )
                            n_pv[0] += 1

                        for jp in range(NHALF):
                            jsl = slice(jp * 128, (jp + 1) * 128)
                            for half in range(2):
                                psl = slice(64 * half, 64 * half + 64)
                                ps = psS.tile([128, 512], F32, tag="st")
                                nc.tensor.matmul(
                                    ps[:],
                                    r32(kT[h][psl, jsl]),
                                    r32(qT[h][psl, isl]),
                                    start=True,
                                    stop=True,
                                    tile_position=(64 * half, 0),
                                )
                                pt = ptp.tile([128, 512], F32R, tag="pt")
                                nc.scalar.activation(pt[:], ps[:], ACTF.Exp)
                                queue.append((pt, jp + half * NHALF))
                            while len(queue) > 2 * SKEW:
                                emit_pv()
                        while queue:
                            emit_pv()

                        rden_f = pCs.tile([1, 512], F32, tag="rden_f")
                        nc.vector.reciprocal(rden_f[:], pso[64:65, :])
                        rden = pCs.tile([1, 512], F32R, tag="rden")
                        nc.vector.tensor_copy(rden[:], rden_f[:])
                        bc = psBC.tile([64, 512], F32, tag="bc")
                        nc.tensor.matmul(
                            bc[:], r32(ones64[:]), r32(rden[:]), start=True, stop=True
                        )
                        osb = pCs.tile([64, 512], F32, tag="osb")
                        nc.vector.tensor_copy(osb[:], pso[0:64, :])
                        if h == 0:
                            nc.vector.tensor_mul(ao1[0:64, isl], osb[:], bc[:])
                        elif h == 2:
                            nc.vector.tensor_mul(ao2[0:64, isl], osb[:], bc[:])
                        else:
                            stg = pCs.tile([64, 512], F32R, tag="stg")
                            nc.vector.tensor_mul(stg[:], osb[:], bc[:])
                            nc.sync.dma_start(ao1[64:128, isl], stg[:])

            # ================= Phase D: output projection (partial) =========
            with (
                tc.tile_pool(name="pD", bufs=3) as pD,
                tc.tile_pool(name="psD", bufs=4, space="PSUM") as psD,
            ):
                for nb in range(NB):
                    blk = slice(nb * 128, (nb + 1) * 128)
                    stage = pD.tile([128, C], F32, tag="stage")
                    for oc, osz in ((0, 512), (512, 256)):
                        ps = psD.tile([128, osz], F32, tag=f"pd{osz}")
                        nc.tensor.matmul(
                            ps[:],
                            r32(ao1[:, blk]),
                            r32(pw128[:, oc : oc + osz]),
                            start=True,
                            stop=False,
                        )
                        nc.tensor.matmul(
                            ps[:],
                            r32(ao2[0:64, blk]),
                            r32(pw64[0:64, oc : oc + osz]),
                            start=False,
                            stop=True,
                        )
                        nc.any.tensor_copy(stage[:, oc : oc + osz], ps[:])
                    nc.sync.dma_start(out_p[blk, :], stage[:])

    nc.compile()
    return nc


@lru_cache(maxsize=2)
def _built(N):
    nc = build_nc(N)
    return nc


def _prep_inputs(x, qkv_w, q_gamma, q_beta, k_gamma, k_beta, proj_w):
    x = np.asarray(x, np.float32)
    qkv_w = np.asarray(qkv_w, np.float32)
    proj_w = np.asarray(proj_w, np.float32)
    B = x.shape[0]
    xts = [np.ascontiguousarray(x[b].T) for b in range(B)]
    gbs = []
    wqs = []
    pws = []
    for g in range(4):
        r = slice(192 * g, 192 * (g + 1))
        wq_rows = np.concatenate(
            [qkv_w[r], qkv_w[768:1536][r], qkv_w[1536:2304][r]], axis=0
        )
        wqs.append(np.ascontiguousarray(wq_rows.T))
        pws.append(np.ascontiguousarray(proj_w[:, r].T))
        gbs.append(
            np.stack(
                [
                    np.asarray(q_gamma, np.float32) * SCALE,
                    np.asarray(q_beta, np.float32) * SCALE,
                    np.asarray(k_gamma, np.float32),
                    np.asarray(k_beta, np.float32),
                ]
            )
        )
    in_maps = []
    for core in range(8):
        b, g = core // 4, core % 4
        in_maps.append(
            {"x_t": xts[b], "wqkv_t": wqs[g], "projw_t": pws[g], "gb": gbs[g]}
        )
    return in_maps


def run_cores(in_maps, N, trace=False):
    from concourse.bass_utils import run_bass_kernel_spmd

    nc = _built(N)
    res = run_bass_kernel_spmd(nc, in_maps, list(range(8)), trace=trace)
    return res


def kernel(x, qkv_w, q_gamma, q_beta, k_gamma, k_beta, proj_w, proj_b):
    x = np.asarray(x, np.float32)
    N = x.shape[1]
    in_maps = _prep_inputs(x, qkv_w, q_gamma, q_beta, k_gamma, k_beta, proj_w)
    res = run_cores(in_maps, N)
    parts = [np.asarray(r["out_p"], np.float32) for r in res.results]
    out0 = parts[0] + parts[1] + parts[2] + parts[3]
    out1 = parts[4] + parts[5] + parts[6] + parts[7]
    out = np.stack([out0, out1]) + np.asarray(proj_b, np.float32)
    return out.astype(np.float32)


# revision 9
# speedup vs baseline: 1.9388x; 1.9388x over previous
"""Trainium2 Bass kernel for CustomAttention (qkv -> per-head LN on q,k -> SDPA -> proj).

Sharding: 8 cores = 2 batches x 4 head-groups (3 heads each).
Per core: qkv projection for its heads from x[b], full attention per head
(scores computed transposed so no probability-matrix transpose is needed,
softmax denominator folded into the PV matmul as a ones column on V),
then a partial output projection over its 192 channels. Host sums the 4
partials per batch and adds proj_b.
"""

import os
import sys
from functools import lru_cache

import numpy as np

for _p in ("/opt/trn_rl_repo", os.path.expanduser("~/.axon_site/_ro/trn_rl_repo")):
    if os.path.isdir(_p) and _p not in sys.path:
        sys.path.insert(0, _p)

import concourse.bass as bass
import concourse.mybir as mybir
from concourse import bacc
import concourse.tile as tile
from concourse.masks import make_identity

F32 = mybir.dt.float32
F32R = mybir.dt.float32r
BF16 = mybir.dt.bfloat16
ALU = mybir.AluOpType
ACTF = mybir.ActivationFunctionType

H = 3          # heads per core
D = 64         # head dim
C = 768        # model dim
J = 3 * H * D  # qkv rows per core = 576
EPS = 1e-5
SCALE = D ** -0.5

# Phase-C software pipeline skew: PV matmuls run this many exp-tiles behind
# the score matmuls so PE never stalls on the ACT engine.
SKEW = 4


def r32(ap):
    return ap.bitcast(F32R)


def build_nc(N=4096):
    """One-core program; all 8 cores run it SPMD with different input data."""
    NB = N // 128          # n-blocks / j-chunks
    IB = N // 512          # i-blocks
    NHALF = NB // 2        # j-chunk pairs for row-tiled score matmuls

    nc = bacc.Bacc("TRN2", target_bir_lowering=False, debug=False)
    x_t = nc.declare_dram_parameter("x_t", [C, N], F32, isOutput=False)
    wqkv_t = nc.declare_dram_parameter("wqkv_t", [C, J], F32, isOutput=False)
    projw_t = nc.declare_dram_parameter("projw_t", [H * D, C], F32, isOutput=False)
    gb = nc.declare_dram_parameter("gb", [4, D], F32, isOutput=False)
    out_p = nc.declare_dram_parameter("out_p", [N, C], F32, isOutput=True)

    with tile.TileContext(nc) as tc:
        with (
            tc.tile_pool(name="persist", bufs=1) as persist,
            tc.tile_pool(name="weights", bufs=1) as weights,
        ):
            # ---- persistent SBUF tensors ----
            # qT duplicated across both partition halves: rows 0:64 == 64:128
            qT = [persist.tile([128, N], BF16, tag=f"qT{h}", name=f"qT{h}") for h in range(H)]
            # kT stacked: rows 0:64 = j in [0,N/2), rows 64:128 = j in [N/2,N)
            kT = [persist.tile([128, N // 2], BF16, tag=f"kT{h}", name=f"kT{h}") for h in range(H)]
            # V augmented with a ones column (index 64) per j-chunk
            vA = [persist.tile([128, NB, 65], BF16, tag=f"vA{h}", name=f"vA{h}") for h in range(H)]
            # attention output, channel-major: ao1 rows = h0,h1; ao2 rows = h2
            ao1 = persist.tile([128, N], F32R, tag="ao1")
            ao2 = persist.tile([64, N], F32R, tag="ao2")

            ident = persist.tile([128, 128], F32, tag="ident")
            make_identity(nc, ident[:])
            ones64 = persist.tile([1, D], BF16, tag="ones64")
            nc.vector.memset(ones64[:], 1.0)
            for h in range(H):
                nc.vector.memset(vA[h][:, :, 64:65], 1.0)

            wq = weights.tile([128, 6, J], F32R, tag="wqkv")
            nc.sync.dma_start(
                wq[:], wqkv_t.rearrange("(ck p) j -> p ck j", p=128).bitcast(F32R)
            )
            pw128 = weights.tile([128, C], F32R, tag="pw128")
            nc.sync.dma_start(pw128[:], projw_t[0:128, :].bitcast(F32R))
            pw64 = weights.tile([64, C], F32R, tag="pw64")
            nc.sync.dma_start(pw64[:], projw_t[128:192, :].bitcast(F32R))
            # gamma/beta broadcast across partitions: [gq*s, bq*s, gk, bk]
            gbt = weights.tile([128, 4, D], F32, tag="gb")
            epst = weights.tile([128, 1], F32, tag="epst")
            nc.vector.memset(epst[:], EPS)
            nc.sync.dma_start(gbt[:], gb[None, :, :].to_broadcast([128, 4, D]))

            # ================= Phase B: qkv + LN + transpose =================
            with (
                tc.tile_pool(name="pB", bufs=3) as pB,
                tc.tile_pool(name="pBs", bufs=3) as pBs,
                tc.tile_pool(name="psB", bufs=4, space="PSUM") as psB,
                tc.tile_pool(name="psT", bufs=4, space="PSUM") as psT,
            ):
                for nb in range(NB):
                    xt = pB.tile([128, 6, 128], F32R, tag="xt")
                    nc.sync.dma_start(
                        xt[:],
                        x_t.rearrange("(ck p) n -> p ck n", p=128)[
                            :, :, nb * 128 : (nb + 1) * 128
                        ].bitcast(F32R),
                    )
                    qkv = pB.tile([128, J], F32, tag="qkv")
                    for half in range(2):
                        ps = psB.tile([128, 288], F32, tag="qkvps")
                        for ck in range(6):
                            nc.tensor.matmul(
                                ps[:],
                                r32(xt[:, ck, :]),
                                r32(wq[:, ck, half * 288 : (half + 1) * 288]),
                                start=(ck == 0),
                                stop=(ck == 5),
                            )
                        nc.any.tensor_copy(qkv[:, half * 288 : (half + 1) * 288], ps[:])

                    # LayerNorm over d for q (cols 0:192) and k (cols 192:384),
                    # writing a column-duplicated result for the transposes.
                    for t in range(2):  # 0 = q, 1 = k
                        src = qkv[:, t * 192 : (t + 1) * 192]
                        src3 = src.rearrange("p (h d) -> p h d", h=H)
                        s1 = pBs.tile([128, H], F32, tag="s1")
                        nc.vector.tensor_reduce(
                            s1[:], src3, mybir.AxisListType.X, ALU.add
                        )
                        mu = pBs.tile([128, H], F32, tag="mu")
                        nc.vector.tensor_scalar_mul(mu[:], s1[:], 1.0 / D)
                        sq = pBs.tile([128, 192], F32, tag="sq")
                        nc.vector.tensor_mul(sq[:], src, src)
                        s2 = pBs.tile([128, H], F32, tag="s2")
                        nc.vector.tensor_reduce(
                            s2[:],
                            sq.rearrange("p (h d) -> p h d", h=H),
                            mybir.AxisListType.X,
                            ALU.add,
                        )
                        var = pBs.tile([128, H], F32, tag="var")
                        # var = s2/D - mu^2  (computed as (s2/D) - mu*mu)
                        nc.vector.tensor_scalar_mul(var[:], s2[:], 1.0 / D)
                        musq = pBs.tile([128, H], F32, tag="musq")
                        nc.vector.tensor_mul(musq[:], mu[:], mu[:])
                        nc.vector.tensor_sub(var[:], var[:], musq[:])
                        std = pBs.tile([128, H], F32, tag="std")
                        nc.scalar.activation(std[:], var[:], ACTF.Sqrt, bias=epst[:])
                        rstd = pBs.tile([128, H], F32, tag="rstd")
                        nc.vector.reciprocal(rstd[:], std[:])
                        # one Newton step: r <- r*(1.5 - 0.5*(var+eps)*r^2)
                        nr = pBs.tile([128, H], F32, tag="nr")
                        nc.vector.tensor_mul(nr[:], rstd[:], rstd[:])
                        ve = pBs.tile([128, H], F32, tag="ve")
                        nc.vector.tensor_scalar_add(ve[:], var[:], EPS)
                        nc.vector.tensor_mul(nr[:], nr[:], ve[:])
                        nc.vector.tensor_scalar(
                            nr[:], nr[:], -0.5, 1.5, ALU.mult, ALU.add
                        )
                        nc.vector.tensor_mul(rstd[:], rstd[:], nr[:])

                        cs = pBs.tile([128, 192], F32, tag="cs")
                        cs3 = cs.rearrange("p (h d) -> p h d", h=H)
                        nc.vector.tensor_sub(
                            cs3, src3, mu[:, :, None].broadcast_to([128, H, D])
                        )
                        nc.vector.tensor_mul(
                            cs3, cs3, rstd[:, :, None].broadcast_to([128, H, D])
                        )
                        gam = gbt[:, 2 * t, :][:, None, :].broadcast_to([128, H, D])
                        bet = gbt[:, 2 * t + 1, :][:, None, :].broadcast_to([128, H, D])
                        nc.vector.tensor_mul(cs3, cs3, gam)
                        dup = pBs.tile([128, H, 2, D], F32, tag=f"dup{t}")
                        nc.vector.tensor_add(dup[:, :, 0, :], cs3, bet)
                        nc.vector.tensor_add(dup[:, :, 1, :], cs3, bet)

                        # transpose each head's duplicated [128,128] slab
                        dupf = dup[:].rearrange("p h r d -> p (h r d)")
                        for h in range(H):
                            pst = psT.tile([128, 128], F32, tag="pst")
                            nc.tensor.transpose(
                                pst[:], dupf[:, h * 128 : (h + 1) * 128], ident[:]
                            )
                            blk = slice(nb * 128, (nb + 1) * 128)
                            if t == 0:
                                nc.any.tensor_copy(qT[h][:, blk], pst[:])
                            else:
                                jh = nb // NHALF
                                cb = nb % NHALF
                                nc.any.tensor_copy(
                                    kT[h][
                                        64 * jh : 64 * jh + 64,
                                        cb * 128 : (cb + 1) * 128,
                                    ],
                                    pst[64 * jh : 64 * jh + 64, :],
                                )
                    for h in range(H):
                        nc.any.tensor_copy(
                            vA[h][:, nb, 0:64],
                            qkv[:, 384 + h * 64 : 384 + (h + 1) * 64],
                        )

            # ================= Phase C: attention =================
            with (
                tc.tile_pool(name="pt", bufs=SKEW + 2) as ptp,
                tc.tile_pool(name="pCs", bufs=3) as pCs,
                tc.tile_pool(name="psS", bufs=3, space="PSUM") as psS,
                tc.tile_pool(name="psO", bufs=2, space="PSUM") as psO,
            ):
                for h in range(H):
                    for ib in range(IB):
                        isl = slice(ib * 512, (ib + 1) * 512)
                        pso = psO.tile([65, 512], F32, tag="pso")
                        queue = []
                        n_pv = [0]

                        def emit_pv(pso=pso, queue=queue, n_pv=n_pv, h=h):
                            pt_half, jc = queue.pop(0)
                            nc.tensor.matmul(
                                pso[:],
                                vA[h][:, jc, :],
                                pt_half,
                                start=(n_pv[0] == 0),
                                stop=(n_pv[0] == NB - 1),
                            )
                            n_pv[0] += 1

                        for jp in range(NHALF):
                            jsl = slice(jp * 128, (jp + 1) * 128)
                            ps = psS.tile([128, 1024], F32, tag="st")
                            for half in range(2):
                                psl = slice(64 * half, 64 * half + 64)
                                nc.tensor.matmul(
                                    ps[:, 512 * half : 512 * half + 512],
                                    kT[h][psl, jsl],
                                    qT[h][psl, isl],
                                    start=True,
                                    stop=True,
                                    tile_position=(64 * half, 0),
                                )
                            pt = ptp.tile([128, 1024], BF16, tag="pt")
                            nc.scalar.activation(pt[:], ps[:], ACTF.Exp)
                            queue.append((pt[:, 0:512], jp))
                            queue.append((pt[:, 512:1024], jp + NHALF))
                            while len(queue) > 2 * SKEW:
                                emit_pv()
                        while queue:
                            emit_pv()

                        rden_f = pCs.tile([1, 512], F32, tag="rden_f")
                        nc.vector.tensor_copy(rden_f[:], pso[64:65, :])
                        rden = pCs.tile([1, 512], F32, tag="rden")
                        nc.vector.reciprocal_approx_fast(rden[:], rden_f[:])
                        rb = pCs.tile([64, 512], F32, tag="rb")
                        nc.gpsimd.partition_broadcast(rb[:], rden[:])
                        if h == 0:
                            nc.vector.tensor_mul(ao1[0:64, isl], pso[0:64, :], rb[:])
                        elif h == 2:
                            nc.vector.tensor_mul(ao2[0:64, isl], pso[0:64, :], rb[:])
                        else:
                            stg = pCs.tile([64, 512], F32R, tag="stg")
                            nc.vector.tensor_mul(stg[:], pso[0:64, :], rb[:])
                            nc.sync.dma_start(ao1[64:128, isl], stg[:])

            # ================= Phase D: output projection (partial) =========
            with (
                tc.tile_pool(name="pD", bufs=3) as pD,
                tc.tile_pool(name="psD", bufs=4, space="PSUM") as psD,
            ):
                for nb in range(NB):
                    blk = slice(nb * 128, (nb + 1) * 128)
                    stage = pD.tile([128, C], F32, tag="stage")
                    for oc, osz in ((0, 512), (512, 256)):
                        ps = psD.tile([128, osz], F32, tag=f"pd{osz}")
                        nc.tensor.matmul(
                            ps[:],
                            r32(ao1[:, blk]),
                            r32(pw128[:, oc : oc + osz]),
                            start=True,
                            stop=False,
                        )
                        nc.tensor.matmul(
                            ps[:],
                            r32(ao2[0:64, blk]),
                            r32(pw64[0:64, oc : oc + osz]),
                            start=False,
                            stop=True,
                        )
                        nc.any.tensor_copy(stage[:, oc : oc + osz], ps[:])
                    nc.sync.dma_start(out_p[blk, :], stage[:])

    nc.compile()
    return nc


@lru_cache(maxsize=2)
def _built(N):
    nc = build_nc(N)
    return nc


def _prep_inputs(x, qkv_w, q_gamma, q_beta, k_gamma, k_beta, proj_w):
    x = np.asarray(x, np.float32)
    qkv_w = np.asarray(qkv_w, np.float32)
    proj_w = np.asarray(proj_w, np.float32)
    B = x.shape[0]
    xts = [np.ascontiguousarray(x[b].T) for b in range(B)]
    gbs = []
    wqs = []
    pws = []
    for g in range(4):
        r = slice(192 * g, 192 * (g + 1))
        wq_rows = np.concatenate(
            [qkv_w[r], qkv_w[768:1536][r], qkv_w[1536:2304][r]], axis=0
        )
        wqs.append(np.ascontiguousarray(wq_rows.T))
        pws.append(np.ascontiguousarray(proj_w[:, r].T))
        gbs.append(
            np.stack(
                [
                    np.asarray(q_gamma, np.float32) * SCALE,
                    np.asarray(q_beta, np.float32) * SCALE,
                    np.asarray(k_gamma, np.float32),
                    np.asarray(k_beta, np.float32),
                ]
            )
        )
    in_maps = []
    for core in range(8):
        b, g = core // 4, core % 4
        in_maps.append(
            {"x_t": xts[b], "wqkv_t": wqs[g], "projw_t": pws[g], "gb": gbs[g]}
        )
    return in_maps


def run_cores(in_maps, N, trace=False):
    from concourse.bass_utils import run_bass_kernel_spmd

    nc = _built(N)
    res = run_bass_kernel_spmd(nc, in_maps, list(range(8)), trace=trace)
    return res


def kernel(x, qkv_w, q_gamma, q_beta, k_gamma, k_beta, proj_w, proj_b):
    x = np.asarray(x, np.float32)
    N = x.shape[1]
    in_maps = _prep_inputs(x, qkv_w, q_gamma, q_beta, k_gamma, k_beta, proj_w)
    res = run_cores(in_maps, N)
    parts = [np.asarray(r["out_p"], np.float32) for r in res.results]
    out0 = parts[0] + parts[1] + parts[2] + parts[3]
    out1 = parts[4] + parts[5] + parts[6] + parts[7]
    out = np.stack([out0, out1]) + np.asarray(proj_b, np.float32)
    return out.astype(np.float32)


# revision 12
# speedup vs baseline: 2.0864x; 1.0761x over previous
"""Trainium2 Bass kernel for CustomAttention (qkv -> per-head LN on q,k -> SDPA -> proj).

Sharding: 8 cores = 2 batches x 4 head-groups (3 heads each).
Per core: qkv projection for its heads from x[b], full attention per head
(scores computed transposed so no probability-matrix transpose is needed,
softmax denominator folded into the PV matmul as a ones column on V),
then a partial output projection over its 192 channels. Host sums the 4
partials per batch and adds proj_b.
"""

import os
import sys
from functools import lru_cache

import numpy as np

for _p in ("/opt/trn_rl_repo", os.path.expanduser("~/.axon_site/_ro/trn_rl_repo")):
    if os.path.isdir(_p) and _p not in sys.path:
        sys.path.insert(0, _p)

import concourse.bass as bass
import concourse.mybir as mybir
from concourse import bacc
import concourse.tile as tile
from concourse.masks import make_identity

F32 = mybir.dt.float32
F32R = mybir.dt.float32r
BF16 = mybir.dt.bfloat16
ALU = mybir.AluOpType
ACTF = mybir.ActivationFunctionType

H = 3          # heads per core
D = 64         # head dim
C = 768        # model dim
J = 3 * H * D  # qkv rows per core = 576
EPS = 1e-5
SCALE = D ** -0.5

# Phase-C software pipeline skew: PV matmuls run this many exp-tiles behind
# the score matmuls so PE never stalls on the ACT engine.
SKEW = 4


def r32(ap):
    return ap.bitcast(F32R)


def build_nc(N=4096):
    """One-core program; all 8 cores run it SPMD with different input data."""
    NB = N // 128          # n-blocks / j-chunks
    IB = N // 512          # i-blocks
    NHALF = NB // 2        # j-chunk pairs for row-tiled score matmuls

    nc = bacc.Bacc("TRN2", target_bir_lowering=False, debug=False)
    x_t = nc.declare_dram_parameter("x_t", [C, N], BF16, isOutput=False)
    wqkv_t = nc.declare_dram_parameter("wqkv_t", [C, J], BF16, isOutput=False)
    projw_t = nc.declare_dram_parameter("projw_t", [H * D, C], F32, isOutput=False)
    gb = nc.declare_dram_parameter("gb", [4, D], F32, isOutput=False)
    out_p = nc.declare_dram_parameter("out_p", [N, C], F32, isOutput=True)

    with tile.TileContext(nc) as tc:
        with (
            tc.tile_pool(name="persist", bufs=1) as persist,
            tc.tile_pool(name="weights", bufs=1) as weights,
        ):
            # ---- persistent SBUF tensors ----
            # qT duplicated across both partition halves: rows 0:64 == 64:128
            qT = [persist.tile([128, N], BF16, tag=f"qT{h}", name=f"qT{h}") for h in range(H)]
            # kT stacked: rows 0:64 = j in [0,N/2), rows 64:128 = j in [N/2,N)
            kT = [persist.tile([128, N // 2], BF16, tag=f"kT{h}", name=f"kT{h}") for h in range(H)]
            # V augmented with a ones column (index 64) per j-chunk
            vA = [persist.tile([128, NB, 65], BF16, tag=f"vA{h}", name=f"vA{h}") for h in range(H)]
            # attention output, channel-major: ao1 rows = h0,h1; ao2 rows = h2
            ao1 = persist.tile([128, N], F32R, tag="ao1")
            ao2 = persist.tile([64, N], F32R, tag="ao2")

            ident = persist.tile([128, 128], F32, tag="ident")
            make_identity(nc, ident[:])
            identb = persist.tile([128, 128], BF16, tag="identb")
            nc.vector.tensor_copy(identb[:], ident[:])
            ones64 = persist.tile([1, D], BF16, tag="ones64")
            nc.vector.memset(ones64[:], 1.0)
            for h in range(H):
                nc.vector.memset(vA[h][:, :, 64:65], 1.0)

            wq = weights.tile([128, 6, J], BF16, tag="wqkv")
            nc.sync.dma_start(
                wq[:], wqkv_t.rearrange("(ck p) j -> p ck j", p=128)
            )
            pw128 = weights.tile([128, C], F32R, tag="pw128")
            nc.sync.dma_start(pw128[:], projw_t[0:128, :].bitcast(F32R))
            pw64 = weights.tile([64, C], F32R, tag="pw64")
            nc.sync.dma_start(pw64[:], projw_t[128:192, :].bitcast(F32R))
            # gamma/beta broadcast across partitions: [gq*s, bq*s, gk, bk]
            gbt = weights.tile([128, 4, D], F32, tag="gb")
            epst = weights.tile([128, 1], F32, tag="epst")
            nc.vector.memset(epst[:], EPS)
            nc.sync.dma_start(gbt[:], gb[None, :, :].to_broadcast([128, 4, D]))

            # ================= Phase B: qkv + LN + transpose =================
            with (
                tc.tile_pool(name="pB", bufs=3) as pB,
                tc.tile_pool(name="pBs", bufs=3) as pBs,
                tc.tile_pool(name="psB", bufs=4, space="PSUM") as psB,
                tc.tile_pool(name="psT", bufs=4, space="PSUM") as psT,
            ):
                for nb in range(NB):
                    xt = pB.tile([128, 6, 128], BF16, tag="xt")
                    nc.sync.dma_start(
                        xt[:],
                        x_t.rearrange("(ck p) n -> p ck n", p=128)[
                            :, :, nb * 128 : (nb + 1) * 128
                        ],
                    )
                    qkv = pB.tile([128, J], F32, tag="qkv")
                    for half in range(2):
                        ps = psB.tile([128, 288], F32, tag="qkvps")
                        for ck in range(6):
                            nc.tensor.matmul(
                                ps[:],
                                xt[:, ck, :],
                                wq[:, ck, half * 288 : (half + 1) * 288],
                                start=(ck == 0),
                                stop=(ck == 5),
                            )
                        nc.any.tensor_copy(qkv[:, half * 288 : (half + 1) * 288], ps[:])

                    # LayerNorm over d for q (cols 0:192) and k (cols 192:384),
                    # writing a column-duplicated result for the transposes.
                    for t in range(2):  # 0 = q, 1 = k
                        src = qkv[:, t * 192 : (t + 1) * 192]
                        src3 = src.rearrange("p (h d) -> p h d", h=H)
                        s1 = pBs.tile([128, H], F32, tag="s1")
                        nc.vector.tensor_reduce(
                            s1[:], src3, mybir.AxisListType.X, ALU.add
                        )
                        mu = pBs.tile([128, H], F32, tag="mu")
                        nc.vector.tensor_scalar_mul(mu[:], s1[:], 1.0 / D)
                        sq = pBs.tile([128, 192], F32, tag="sq")
                        nc.vector.tensor_mul(sq[:], src, src)
                        s2 = pBs.tile([128, H], F32, tag="s2")
                        nc.vector.tensor_reduce(
                            s2[:],
                            sq.rearrange("p (h d) -> p h d", h=H),
                            mybir.AxisListType.X,
                            ALU.add,
                        )
                        var = pBs.tile([128, H], F32, tag="var")
                        # var = s2/D - mu^2  (computed as (s2/D) - mu*mu)
                        nc.vector.tensor_scalar_mul(var[:], s2[:], 1.0 / D)
                        musq = pBs.tile([128, H], F32, tag="musq")
                        nc.vector.tensor_mul(musq[:], mu[:], mu[:])
                        nc.vector.tensor_sub(var[:], var[:], musq[:])
                        std = pBs.tile([128, H], F32, tag="std")
                        nc.scalar.activation(std[:], var[:], ACTF.Sqrt, bias=epst[:])
                        rstd = pBs.tile([128, H], F32, tag="rstd")
                        nc.vector.reciprocal(rstd[:], std[:])
                        # one Newton step: r <- r*(1.5 - 0.5*(var+eps)*r^2)
                        nr = pBs.tile([128, H], F32, tag="nr")
                        nc.vector.tensor_mul(nr[:], rstd[:], rstd[:])
                        ve = pBs.tile([128, H], F32, tag="ve")
                        nc.vector.tensor_scalar_add(ve[:], var[:], EPS)
                        nc.vector.tensor_mul(nr[:], nr[:], ve[:])
                        nc.vector.tensor_scalar(
                            nr[:], nr[:], -0.5, 1.5, ALU.mult, ALU.add
                        )
                        nc.vector.tensor_mul(rstd[:], rstd[:], nr[:])

                        cs = pBs.tile([128, 192], F32, tag="cs")
                        cs3 = cs.rearrange("p (h d) -> p h d", h=H)
                        nc.vector.tensor_sub(
                            cs3, src3, mu[:, :, None].broadcast_to([128, H, D])
                        )
                        nc.vector.tensor_mul(
                            cs3, cs3, rstd[:, :, None].broadcast_to([128, H, D])
                        )
                        gam = gbt[:, 2 * t, :][:, None, :].broadcast_to([128, H, D])
                        bet = gbt[:, 2 * t + 1, :][:, None, :].broadcast_to([128, H, D])
                        nc.vector.tensor_mul(cs3, cs3, gam)
                        dup = pBs.tile([128, H, 2, D], BF16, tag=f"dup{t}")
                        nc.vector.tensor_add(dup[:, :, 0, :], cs3, bet)
                        nc.vector.tensor_add(dup[:, :, 1, :], cs3, bet)

                        # transpose each head's duplicated [128,128] slab
                        dupf = dup[:].rearrange("p h r d -> p (h r d)")
                        for h in range(H):
                            pst = psT.tile([128, 128], BF16, tag="pst")
                            nc.tensor.transpose(
                                pst[:], dupf[:, h * 128 : (h + 1) * 128], identb[:]
                            )
                            blk = slice(nb * 128, (nb + 1) * 128)
                            if t == 0:
                                nc.any.tensor_copy(qT[h][:, blk], pst[:])
                            else:
                                jh = nb // NHALF
                                cb = nb % NHALF
                                nc.any.tensor_copy(
                                    kT[h][
                                        64 * jh : 64 * jh + 64,
                                        cb * 128 : (cb + 1) * 128,
                                    ],
                                    pst[64 * jh : 64 * jh + 64, :],
                                )
                    for h in range(H):
                        nc.any.tensor_copy(
                            vA[h][:, nb, 0:64],
                            qkv[:, 384 + h * 64 : 384 + (h + 1) * 64],
                        )

            # ================= Phase C: attention =================
            with (
                tc.tile_pool(name="pt", bufs=SKEW + 2) as ptp,
                tc.tile_pool(name="pCs", bufs=3) as pCs,
                tc.tile_pool(name="pD", bufs=3) as pD,
                tc.tile_pool(name="psS", bufs=2, space="PSUM") as psS,
                tc.tile_pool(name="psO", bufs=2, space="PSUM") as psO,
                tc.tile_pool(name="psD", bufs=1, space="PSUM") as psD,
            ):
                for ib in range(IB):
                    for h in range(H):
                        isl = slice(ib * 512, (ib + 1) * 512)
                        pso = psO.tile([65, 512], F32, tag="pso")
                        queue = []
                        n_pv = [0]

                        def emit_pv(pso=pso, queue=queue, n_pv=n_pv, h=h):
                            pt_half, jc = queue.pop(0)
                            nc.tensor.matmul(
                                pso[:],
                                vA[h][:, jc, :],
                                pt_half,
                                start=(n_pv[0] == 0),
                                stop=(n_pv[0] == NB - 1),
                            )
                            n_pv[0] += 1

                        for jp in range(NHALF):
                            jsl = slice(jp * 128, (jp + 1) * 128)
                            ps = psS.tile([128, 1024], F32, tag="st")
                            for half in range(2):
                                psl = slice(64 * half, 64 * half + 64)
                                nc.tensor.matmul(
                                    ps[:, 512 * half : 512 * half + 512],
                                    kT[h][psl, jsl],
                                    qT[h][psl, isl],
                                    start=True,
                                    stop=True,
                                    tile_position=(64 * half, 0),
                                )
                            pt = ptp.tile([128, 1024], BF16, tag="pt")
                            nc.scalar.activation(pt[:], ps[:], ACTF.Exp)
                            queue.append((pt[:, 0:512], jp))
                            queue.append((pt[:, 512:1024], jp + NHALF))
                            while len(queue) > 2 * SKEW:
                                emit_pv()
                        while queue:
                            emit_pv()

                        rden_f = pCs.tile([1, 512], F32, tag="rden_f")
                        nc.vector.tensor_copy(rden_f[:], pso[64:65, :])
                        rden = pCs.tile([1, 512], F32, tag="rden")
                        nc.vector.reciprocal_approx_fast(rden[:], rden_f[:])
                        rb = pCs.tile([64, 512], F32, tag="rb")
                        nc.gpsimd.partition_broadcast(rb[:], rden[:])
                        if h == 0:
                            nc.vector.tensor_mul(ao1[0:64, isl], pso[0:64, :], rb[:])
                        elif h == 2:
                            nc.vector.tensor_mul(ao2[0:64, isl], pso[0:64, :], rb[:])
                        else:
                            stg = pCs.tile([64, 512], F32R, tag="stg")
                            nc.vector.tensor_mul(stg[:], pso[0:64, :], rb[:])
                            nc.sync.dma_start(ao1[64:128, isl], stg[:])
                    for nb in range(ib * 4, ib * 4 + 4):
                        blk = slice(nb * 128, (nb + 1) * 128)
                        stage = pD.tile([128, C], F32, tag="stage")
                        for oc, osz in ((0, 512), (512, 256)):
                            ps = psD.tile([128, osz], F32, tag=f"pd{osz}")
                            nc.tensor.matmul(
                                ps[:],
                                r32(ao1[:, blk]),
                                r32(pw128[:, oc : oc + osz]),
                                start=True,
                                stop=False,
                            )
                            nc.tensor.matmul(
                                ps[:],
                                r32(ao2[0:64, blk]),
                                r32(pw64[0:64, oc : oc + osz]),
                                start=False,
                                stop=True,
                            )
                            nc.any.tensor_copy(stage[:, oc : oc + osz], ps[:])
                        nc.sync.dma_start(out_p[blk, :], stage[:])


    nc.compile()
    return nc


@lru_cache(maxsize=2)
def _built(N):
    nc = build_nc(N)
    return nc


def _prep_inputs(x, qkv_w, q_gamma, q_beta, k_gamma, k_beta, proj_w):
    x = np.asarray(x, np.float32)
    qkv_w = np.asarray(qkv_w, np.float32)
    proj_w = np.asarray(proj_w, np.float32)
    B = x.shape[0]
    import ml_dtypes
    xts = [np.ascontiguousarray(x[b].T).astype(ml_dtypes.bfloat16) for b in range(B)]
    gbs = []
    wqs = []
    pws = []
    for g in range(4):
        r = slice(192 * g, 192 * (g + 1))
        wq_rows = np.concatenate(
            [qkv_w[r], qkv_w[768:1536][r], qkv_w[1536:2304][r]], axis=0
        )
        import ml_dtypes as _md
        wqs.append(np.ascontiguousarray(wq_rows.T).astype(_md.bfloat16))
        pws.append(np.ascontiguousarray(proj_w[:, r].T))
        gbs.append(
            np.stack(
                [
                    np.asarray(q_gamma, np.float32) * SCALE,
                    np.asarray(q_beta, np.float32) * SCALE,
                    np.asarray(k_gamma, np.float32),
                    np.asarray(k_beta, np.float32),
                ]
            )
        )
    in_maps = []
    for core in range(8):
        b, g = core // 4, core % 4
        in_maps.append(
            {"x_t": xts[b], "wqkv_t": wqs[g], "projw_t": pws[g], "gb": gbs[g]}
        )
    return in_maps


def run_cores(in_maps, N, trace=False):
    from concourse.bass_utils import run_bass_kernel_spmd

    nc = _built(N)
    res = run_bass_kernel_spmd(nc, in_maps, list(range(8)), trace=trace)
    return res


def kernel(x, qkv_w, q_gamma, q_beta, k_gamma, k_beta, proj_w, proj_b):
    x = np.asarray(x, np.float32)
    N = x.shape[1]
    in_maps = _prep_inputs(x, qkv_w, q_gamma, q_beta, k_gamma, k_beta, proj_w)
    res = run_cores(in_maps, N)
    parts = [np.asarray(r["out_p"], np.float32) for r in res.results]
    out0 = parts[0] + parts[1] + parts[2] + parts[3]
    out1 = parts[4] + parts[5] + parts[6] + parts[7]
    out = np.stack([out0, out1]) + np.asarray(proj_b, np.float32)
    return out.astype(np.float32)
